# revision 1
# baseline (speedup 1.0000x reference)
"""BasicRGCN Trainium2 kernel — 8-core SPMD Bass/Tile implementation.

Model (PyG-style RGCNConv x2 + global_mean_pool):
  h1 = relu(x @ root1 + b1 + sum_r mean_r(x_src) @ W1[r])
  h2 = relu(h1 @ root2 + b2 + sum_r mean_r(h1_src) @ W2[r])
  out[g] = mean over nodes in graph g of h2            -> [64, 128] f32

Distribution: nodes (and their incoming edges) are sharded over 8 cores by
destination id (12544 nodes/core). Per-relation weights are replicated.
Layer-1 edge features (15-dim x rows, scaled by 1/deg) are pre-gathered on
the host as part of input sharding. Layer-2 features (h1, device-computed)
are exchanged with an AllGather and gathered on-device via indirect DMA.
Aggregation uses a one-hot matmul: for each 128-edge group, a bf16
selection matrix built on the vector engine (iota + is_equal against
relation-folded window keys) scatters gathered rows into per-window PSUM
accumulators on the tensor engine, which also handles duplicate
destinations for free.
"""
import sys
sys.path.insert(0, "/opt/trn_rl_repo")
import numpy as np

import concourse.bass as bass
import concourse.mybir as mybir
import concourse.tile as tile_mod
from concourse.tile import TileContext
from concourse.bacc import Bacc
from concourse.ap import AP
from concourse.masks import make_identity
from concourse.tile_rust import add_dep_helper

# ---------------------------------------------------------------- constants
NCORES = 8
N = 100000
NPAD = 100352            # 8 * 12544
PC = NPAD // NCORES      # 12544 nodes per core
W = 32                   # dst window width (4W = 128 one-hot columns)
NW = PC // W             # 392 windows per core
TW = 8                   # windows per output tile (256 nodes)
NT = NW // TW            # 49 output tiles
H = 128                  # hidden dim
F1 = 16                  # padded layer-1 input dim (15 real)
R = 4                    # relations
NGRAPH = 64

_bf16 = mybir.dt.bfloat16
_f32 = mybir.dt.float32


def _to_bf16(a):
    """f32 -> bf16 (round-to-nearest-even) stored as numpy uint16 view array."""
    import ml_dtypes
    return a.astype(ml_dtypes.bfloat16)


# ------------------------------------------------------- tile/walrus patches
def _patch_tile_drain():
    """This deployment's walrus accepts only ONE sync-wait per instruction:
    split the end-of-TileContext drain into single-wait drains."""
    def _patched(self, tick_clock, wait_clock):
        nc = self.nc
        drain_inst = nc.sync.drain()
        wait_clock.add_sem_waits(
            drain_inst.ins, tile_mod.ScopedClock({None: tick_clock.global_clock})
        )
        si = drain_inst.ins.sync_info
        if si is not None and si.on_wait and len(si.on_wait) > 1:
            waits = list(si.on_wait)
            si.on_wait = waits[:1]
            for i in range(1, len(waits)):
                extra = nc.sync.drain()
                esi = extra.ins.sync_info
                if esi is None:
                    extra.ins.sync_info = mybir.SyncInfo(
                        on_wait=[waits[i]], on_update=[])
                else:
                    esi.on_wait = [waits[i]]
        nc.all_engine_barrier()
        assert self.sems is not None
        popped = nc._tile_sem_poison_stack.pop()
        assert popped is self._sem_poison
        nc.clear_and_free_semaphores(list(self.sems.allocated().values()))
        nc.all_engine_barrier()
    TileContext._drain_and_barrier = _patched


_patch_tile_drain()
_legal_ctr = [0]


def _legalize_waits(nc, maxw=1):
    """Split >maxw sync-waits on any instruction onto preceding same-engine
    NoOps (engine streams are in-order, so this is semantics-preserving)."""
    for f in nc.m.functions:
        for blk in f.blocks:
            insts = list(blk.instructions)
            out = []
            changed = False
            for ins in insts:
                si = ins.sync_info
                if si is not None and si.on_wait and len(si.on_wait) > maxw:
                    waits = list(si.on_wait)
                    for i in range(0, len(waits) - maxw, maxw):
                        _legal_ctr[0] += 1
                        nop = mybir.InstNoOp(
                            name=f"legalw-{_legal_ctr[0]}", ins=[], outs=[])
                        nop.engine = ins.engine
                        nop.sync_info = mybir.SyncInfo(
                            on_wait=waits[i:i + maxw], on_update=[])
                        out.append(nop)
                    si.on_wait = waits[len(waits) - maxw:]
                    changed = True
                out.append(ins)
            if changed:
                blk.instructions = out


# ------------------------------------------------------------- host prep
def _host_prep(x, W1, root1, b1, W2, root2, b2, edge_index, edge_type, batch):
    """Shard/repack all inputs. Returns (per_core_inmaps, host_ctx)."""
    src = np.asarray(edge_index[0], dtype=np.int64)
    dst = np.asarray(edge_index[1], dtype=np.int64)
    rel = np.asarray(edge_type, dtype=np.int64)
    batch = np.asarray(batch, dtype=np.int64)
    x = np.asarray(x, dtype=np.float32)
    E = src.shape[0]

    # per-(relation, dst) in-degree counts -> mean scale
    cnt = np.zeros((R, N), dtype=np.int64)
    np.add.at(cnt, (rel, dst), 1)
    recip = (1.0 / np.maximum(cnt, 1)).astype(np.float32)   # [R, N]

    core_of = dst // PC
    woff = dst % PC
    win = woff // W
    key = rel * W + (woff % W)                               # [0, 4W)

    # per-(core, window) capacities shared by all cores (SPMD uniformity)
    cw = np.zeros((NCORES, NW), dtype=np.int64)
    np.add.at(cw, (core_of, win), 1)
    cap = np.maximum(np.ceil(cw.max(axis=0) / 128).astype(np.int64), 1)  # [NW]
    g_off = np.concatenate([[0], np.cumsum(cap)])            # [NW+1]
    TOTAL_G = int(g_off[-1])

    # slot assignment: sort edges by (core, window); edges of (k,w) fill
    # slots g_off[w]*128 ... in order
    order = np.lexsort((win, core_of))
    s_src, s_rel, s_dst = src[order], rel[order], dst[order]
    s_core, s_win = core_of[order], win[order]
    s_key = key[order]
    s_scale = recip[s_rel, s_dst].astype(np.float32)

    NSLOT = TOTAL_G * 128
    keys_all = np.full((NCORES, NSLOT), -1.0, dtype=np.float32)
    offs_all = np.full((NCORES, NSLOT), NPAD - 1, dtype=np.int32)
    xsl_all = np.zeros((NCORES, NSLOT, F1), dtype=np.float32)

    # position within (core, window)
    cell_id = s_core * NW + s_win
    cell_start = np.zeros(NCORES * NW + 1, dtype=np.int64)
    np.add.at(cell_start, cell_id + 1, 1)
    cell_start = np.cumsum(cell_start)
    pos_in_cell = np.arange(E) - cell_start[cell_id]
    slot = (g_off[s_win] * 128 + pos_in_cell).astype(np.int64)

    keys_all[s_core, slot] = s_key
    offs_all[s_core, slot] = s_src.astype(np.int32)
    xsl_all[s_core, slot, :15] = x[s_src] * s_scale[:, None]

    # [core, 128, TOTAL_G] layouts (slot = g*128 + p  ->  [p, g])
    keys_pg = _to_bf16(
        keys_all.reshape(NCORES, TOTAL_G, 128).transpose(0, 2, 1).copy())
    offs_pg = offs_all.reshape(NCORES, TOTAL_G, 128).transpose(0, 2, 1).copy()
    xsl_pg = _to_bf16(
        xsl_all.reshape(NCORES, TOTAL_G, 128, F1).transpose(0, 2, 1, 3).copy())

    # scale columns for layer 2 (partition-replicated): col (w, r*W+o)
    sc = np.zeros((NCORES, NW * 4 * W), dtype=np.float32)
    node_ids = (np.arange(NPAD) % PC)
    for k in range(NCORES):
        nd = k * PC + np.arange(PC)         # global node ids of this core
        nd = np.minimum(nd, N - 1)          # pad nodes: value irrelevant
        r_grid = np.repeat(np.arange(R), W)  # [4W] -> r
        o_grid = np.tile(np.arange(W), R)    # [4W] -> offset
        # [NW, 4W]
        nodes_w = (k * PC + (np.arange(NW)[:, None] * W + o_grid[None, :]))
        nodes_w = np.minimum(nodes_w, N - 1)
        sc[k] = recip[r_grid[None, :], nodes_w].reshape(-1)
    sc_bf = _to_bf16(sc)                     # [NCORES, NW*128]
    sc_rep = np.broadcast_to(
        sc_bf[:, None, :], (NCORES, 128, NW * 4 * W)).copy()

    # batch one-hot [PC, 64] per core
    bone = np.zeros((NCORES, PC, NGRAPH), dtype=np.float32)
    for k in range(NCORES):
        nd = k * PC + np.arange(PC)
        real = nd < N
        bone[k, real, batch[nd[real]]] = 1.0
    bone_bf = _to_bf16(bone)

    # x^T own block, padded feat rows [128, PC]
    xT = np.zeros((NCORES, 128, PC), dtype=np.float32)
    for k in range(NCORES):
        nd = k * PC + np.arange(PC)
        real = nd < N
        xT[k][:15][:, real] = x[nd[real]].T
    xT_bf = _to_bf16(xT)

    # weights (replicated)
    def padw(w, rows):
        out = np.zeros((128, H), dtype=np.float32)
        out[:rows] = w
        return _to_bf16(out)

    W1p = np.stack([padw(np.asarray(W1)[r], 15) for r in range(R)])   # [4,128,128]
    root1p = padw(np.asarray(root1), 15)
    W2p = np.stack([padw(np.asarray(W2)[r], H) for r in range(R)])
    root2p = padw(np.asarray(root2), H)
    b1f = np.asarray(b1, dtype=np.float32).reshape(H, 1)
    b2f = np.asarray(b2, dtype=np.float32).reshape(H, 1)

    in_maps = []
    for k in range(NCORES):
        in_maps.append({
            "xsl": xsl_pg[k],          # [128, TOTAL_G, F1] bf16
            "keys": keys_pg[k],        # [128, TOTAL_G] bf16
            "offs": offs_pg[k],        # [128, TOTAL_G] int32
            "screp": sc_rep[k],        # [128, NW*128] bf16
            "bone": bone_bf[k],        # [PC, 64] bf16
            "xT": xT_bf[k],            # [128, PC] bf16
            "W1p": W1p, "root1p": root1p,
            "W2p": W2p, "root2p": root2p,
            "b1": b1f, "b2": b2f,
        })

    gcounts = np.maximum(np.bincount(batch, minlength=NGRAPH), 1).astype(np.float32)
    host_ctx = {"cap": cap, "g_off": g_off, "TOTAL_G": TOTAL_G,
                "gcounts": gcounts}
    return in_maps, host_ctx


# ------------------------------------------------------------- device build
def _build_nc(cap, g_off, TOTAL_G, legalize=True):
    nc = Bacc("TRN2", num_devices=NCORES)
    xsl = nc.dram_tensor("xsl", [128, TOTAL_G, F1], _bf16, kind="ExternalInput")
    keys = nc.dram_tensor("keys", [128, TOTAL_G], _bf16, kind="ExternalInput")
    offs = nc.dram_tensor("offs", [128, TOTAL_G], mybir.dt.int32, kind="ExternalInput")
    screp = nc.dram_tensor("screp", [128, NW * 128], _bf16, kind="ExternalInput")
    bone = nc.dram_tensor("bone", [PC, NGRAPH], _bf16, kind="ExternalInput")
    xT = nc.dram_tensor("xT", [128, PC], _bf16, kind="ExternalInput")
    W1p = nc.dram_tensor("W1p", [R, 128, H], _bf16, kind="ExternalInput")
    root1p = nc.dram_tensor("root1p", [128, H], _bf16, kind="ExternalInput")
    W2p = nc.dram_tensor("W2p", [R, 128, H], _bf16, kind="ExternalInput")
    root2p = nc.dram_tensor("root2p", [128, H], _bf16, kind="ExternalInput")
    b1 = nc.dram_tensor("b1", [H, 1], _f32, kind="ExternalInput")
    b2 = nc.dram_tensor("b2", [H, 1], _f32, kind="ExternalInput")
    h1own = nc.dram_tensor("h1own", [PC, H], _bf16, kind="Internal")
    h1tab = nc.dram_tensor("h1tab", [NPAD, H], _bf16, kind="Internal",
                           addr_space="Shared")
    pool_out = nc.dram_tensor("pool_out", [NGRAPH, H], _f32, kind="ExternalOutput")

    CAPMAX = int(cap.max())

    with TileContext(nc, num_cores=NCORES) as tc:
        import contextlib
        with contextlib.ExitStack() as ctx:
            const_p = ctx.enter_context(tc.tile_pool(name="const", bufs=1))
            wpool = ctx.enter_context(tc.tile_pool(name="wts", bufs=1))
            hpool = ctx.enter_context(tc.tile_pool(name="hT", bufs=1))
            feed_p = ctx.enter_context(tc.tile_pool(name="feed", bufs=3))
            oh_p = ctx.enter_context(tc.tile_pool(name="oh", bufs=4))
            g_p = ctx.enter_context(tc.tile_pool(name="gat", bufs=24))
            sb_p = ctx.enter_context(tc.tile_pool(name="stile", bufs=2))
            sc_p = ctx.enter_context(tc.tile_pool(name="sctile", bufs=2))
            off_p = ctx.enter_context(tc.tile_pool(name="offt", bufs=4))
            tok_p = ctx.enter_context(tc.tile_pool(name="tok", bufs=3))
            bo_p = ctx.enter_context(tc.tile_pool(name="bo", bufs=3))
            misc_p = ctx.enter_context(tc.tile_pool(name="misc", bufs=2))
            ps_agg = ctx.enter_context(
                tc.tile_pool(name="ps_agg", bufs=1, space="PSUM"))
            ps_out = ctx.enter_context(
                tc.tile_pool(name="ps_out", bufs=1, space="PSUM"))
            ps_tr = ctx.enter_context(
                tc.tile_pool(name="ps_tr", bufs=2, space="PSUM"))
            ps_pool = ctx.enter_context(
                tc.tile_pool(name="ps_pool", bufs=1, space="PSUM"))

            # constants
            iota_i = const_p.tile([128, CAPMAX * TW * 128], mybir.dt.int32)
            nc.gpsimd.iota(iota_i[:], pattern=[[0, CAPMAX * TW], [1, 128]],
                           base=0, channel_multiplier=0)
            iota_bf = const_p.tile([128, CAPMAX * TW * 128], _bf16)
            nc.vector.tensor_copy(iota_bf[:], iota_i[:])
            ident = const_p.tile([128, 128], _bf16)
            make_identity(nc, ident[:])

            # weights resident in SBUF
            w1t = [wpool.tile([128, H], _bf16, tag=f"w1_{r}", name=f"w1_{r}") for r in range(R)]
            w2t = [wpool.tile([128, H], _bf16, tag=f"w2_{r}", name=f"w2_{r}") for r in range(R)]
            r1t = wpool.tile([128, H], _bf16, tag="r1")
            r2t = wpool.tile([128, H], _bf16, tag="r2")
            b1t = wpool.tile([H, 1], _f32, tag="b1")
            b2t = wpool.tile([H, 1], _f32, tag="b2")
            for r in range(R):
                nc.sync.dma_start(out=w1t[r][:], in_=W1p[r])
                nc.sync.dma_start(out=w2t[r][:], in_=W2p[r])
            nc.sync.dma_start(out=r1t[:], in_=root1p[:, :])
            nc.sync.dma_start(out=r2t[:], in_=root2p[:, :])
            nc.sync.dma_start(out=b1t[:], in_=b1[:, :])
            nc.sync.dma_start(out=b2t[:], in_=b2[:, :])

            hT_x = hpool.tile([128, PC], _bf16, tag="hT_x")     # layer1 rhs
            hT_1 = hpool.tile([128, PC], _bf16, tag="hT_1")     # layer1 out
            nc.sync.dma_start(out=hT_x[:], in_=xT[:, :])

            pool_acc = const_p.tile([NGRAPH, H], _f32)
            nc.vector.memset(pool_acc[:], 0.0)

            def bcast_inner(tile_ap, ncols, inner):
                """[128, ncols] -> AP [128, ncols, inner] (step-0 inner)."""
                base = tile_ap
                newap = [list(base.ap[0]), [base.ap[-1][0], ncols], [0, inner]]
                return AP(base.tensor, base.offset, newap)

            def layer(L, hT_in, wts, roott, bt, hT_out, ag_dep=None):
                ag_dep_done = [False]
                """Emit one RGCN layer. L=1 feeds from host-gathered xsl,
                L=2 indirect-gathers rows of h1tab."""
                for t in range(NT):
                    w0 = t * TW
                    # aggregation PSUM: two banks of 4 windows
                    agg = [ps_agg.tile([128, 512], _f32, tag=f"agg{h}", name=f"agg{h}")
                           for h in range(2)]
                    if L == 2:
                        g0, g1 = int(g_off[w0]), int(g_off[w0 + TW])
                        offt = off_p.tile([128, CAPMAX * TW], mybir.dt.int32,
                                          tag="offt")
                        nc.sync.dma_start(out=offt[:, :g1 - g0],
                                          in_=offs[:, g0:g1])
                    else:
                        g0, g1 = int(g_off[w0]), int(g_off[w0 + TW])
                        feedt = feed_p.tile([128, CAPMAX * TW, F1], _bf16,
                                            tag="feed")
                        nc.sync.dma_start(
                            out=feedt[:, :g1 - g0, :],
                            in_=xsl[:, g0:g1, :])
                    keyt = misc_p.tile([128, CAPMAX * TW], _bf16, tag="keyt")
                    nc.sync.dma_start(out=keyt[:, :g1 - g0], in_=keys[:, g0:g1])
                    # one-hot for all groups of this tile in one DVE op
                    oht = oh_p.tile([128, CAPMAX * TW * 128], _bf16, tag="oht")
                    nglocal = g1 - g0
                    nc.vector.tensor_tensor(
                        out=oht[:, :nglocal * 128],
                        in0=iota_bf[:, :nglocal * 128],
                        in1=bcast_inner(keyt[:, :nglocal], nglocal, 128),
                        op=mybir.AluOpType.is_equal)

                    for wi in range(TW):
                        w = w0 + wi
                        ps = agg[wi // 4]
                        colsl = slice((wi % 4) * 128, (wi % 4) * 128 + 128)
                        ngw = int(cap[w])
                        for j in range(ngw):
                            gg = int(g_off[w]) + j - g0   # local group idx
                            if L == 2:
                                gt = g_p.tile([128, H], _bf16, tag="gt")
                                gi = nc.gpsimd.indirect_dma_start(
                                    out=gt[:, :], out_offset=None,
                                    in_=h1tab[:, :],
                                    in_offset=bass.IndirectOffsetOnAxis(
                                        ap=offt[:, gg:gg + 1], axis=0))
                                if ag_dep is not None and not ag_dep_done[0]:
                                    add_dep_helper(
                                        gi.ins, ag_dep,
                                        reason="L2 gather reads AllGather output")
                                    ag_dep_done[0] = True
                                lhs = gt[:, :]
                            else:
                                lhs = feedt[:, gg, :]
                            M = H if L == 2 else F1
                            nc.tensor.matmul(
                                agg[wi // 4][:M, colsl],
                                lhsT=lhs,
                                rhs=oht[:, gg * 128:(gg + 1) * 128],
                                start=(j == 0), stop=(j == ngw - 1))

                    # scale (L2) / plain (L1) copy PSUM -> SBUF S~^T bf16
                    KF = H if L == 2 else F1
                    st = sb_p.tile([128, TW * 128], _bf16, tag="st")
                    if L == 2:
                        sct = sc_p.tile([128, TW * 128], _bf16, tag="sct")
                        nc.sync.dma_start(
                            out=sct[:], in_=screp[:, w0 * 128:(w0 + TW) * 128])
                        for h in range(2):
                            nc.vector.tensor_tensor(
                                out=st[:, h * 512:(h + 1) * 512],
                                in0=agg[h][:, :], in1=sct[:, h * 512:(h + 1) * 512],
                                op=mybir.AluOpType.mult)
                    else:
                        for h in range(2):
                            nc.vector.tensor_copy(
                                st[:KF, h * 512:(h + 1) * 512],
                                agg[h][:KF, :])

                    # transform: out2^T [128, 256 nodes]
                    op_ps = ps_out.tile([128, 256], _f32, tag="ops")
                    nsl = slice(t * 256, (t + 1) * 256)
                    nc.tensor.matmul(op_ps[:, :], lhsT=roott[:KF, :],
                                     rhs=hT_in[:KF, nsl], start=True, stop=False)
                    st3 = st[:KF, :].rearrange("p (a b) -> p a b", b=128)
                    for r in range(R):
                        nc.tensor.matmul(op_ps[:, :], lhsT=wts[r][:KF, :],
                                         rhs=st3[:, :, r * W:(r + 1) * W],
                                         start=False,
                                         stop=(r == R - 1))
                    # bias + relu -> hT_out (bf16)
                    nc.scalar.activation(
                        out=hT_out[:, nsl], in_=op_ps[:, :],
                        func=mybir.ActivationFunctionType.Relu,
                        bias=bt[:], scale=1.0)

                    # transpose to token-major for h1 table / pooling
                    for half in range(2):
                        tr = ps_tr.tile([128, 128], _bf16, tag="tr")
                        nc.tensor.transpose(
                            tr[:, :],
                            hT_out[:, t * 256 + half * 128: t * 256 + (half + 1) * 128],
                            ident[:])
                        tok = tok_p.tile([128, 128], _bf16, tag="tok")
                        nc.scalar.activation(
                            out=tok[:], in_=tr[:, :],
                            func=mybir.ActivationFunctionType.Copy)
                        row0 = t * 256 + half * 128
                        if L == 1:
                            nc.sync.dma_start(
                                out=h1own[row0:row0 + 128, :], in_=tok[:])
                        else:
                            bt_t = bo_p.tile([128, NGRAPH], _bf16, tag="bt")
                            nc.sync.dma_start(
                                out=bt_t[:], in_=bone[row0:row0 + 128, :])
                            pp = ps_pool.tile([NGRAPH, H], _f32, tag="pp")
                            nc.tensor.matmul(pp[:, :], lhsT=bt_t[:],
                                             rhs=tok[:], start=True, stop=True)
                            nc.vector.tensor_tensor(
                                out=pool_acc[:], in0=pool_acc[:], in1=pp[:, :],
                                op=mybir.AluOpType.add)

            layer(1, hT_x, w1t, r1t, b1t, hT_1)
            ag = nc.gpsimd.collective_compute(
                "AllGather", mybir.AluOpType.bypass,
                replica_groups=[list(range(NCORES))],
                ins=[h1own[:, :]], outs=[h1tab[:, :]])
            ag_inst = ag.ins if hasattr(ag, "ins") else ag
            hT_2 = hpool.tile([128, PC], _bf16, tag="hT_2")
            layer(2, hT_1, w2t, r2t, b2t, hT_2, ag_dep=ag_inst)

            nc.sync.dma_start(out=pool_out[:, :], in_=pool_acc[:])

    nc.finalize()
    if legalize:
        _legalize_waits(nc)
    return nc


# ------------------------------------------------------------- runner
_CACHE = {}


def _get_compiled(cap, g_off, TOTAL_G):
    key = ("nc", TOTAL_G, tuple(cap.tolist()))
    if key not in _CACHE:
        import jax
        from jax.sharding import Mesh, PartitionSpec
        from jax.experimental.shard_map import shard_map
        from concourse.bass2jax import (
            _bass_exec_p, partition_id_tensor, install_neuronx_cc_hook)
        install_neuronx_cc_hook()
        nc = _build_nc(cap, g_off, TOTAL_G)

        partition_name = (nc.partition_id_tensor.name
                          if nc.partition_id_tensor else None)
        in_names, out_names, out_avals = [], [], []
        for alloc in nc.m.functions[0].allocations:
            if not isinstance(alloc, mybir.MemoryLocationSet):
                continue
            name = alloc.memorylocations[0].name
            if alloc.kind == "ExternalInput":
                if name != partition_name and name != (
                        nc.dbg_addr.name if nc.dbg_addr is not None else None):
                    in_names.append(name)
            elif alloc.kind == "ExternalOutput":
                out_names.append(name)
                out_avals.append(jax.core.ShapedArray(
                    tuple(alloc.tensor_shape), mybir.dt.np(alloc.dtype)))
        n_params, n_outs = len(in_names), len(out_names)
        all_in = list(in_names) + list(out_names)
        if nc.dbg_addr is not None:
            all_in.append(nc.dbg_addr.name)
        if partition_name is not None:
            all_in.append(partition_name)

        def _body(*args):
            operands = list(args)
            if nc.dbg_addr is not None:
                operands.append(jax.numpy.zeros((1, 2), jax.numpy.uint32))
            if partition_name is not None:
                operands.append(partition_id_tensor())
            outs = _bass_exec_p.bind(
                *operands, out_avals=tuple(out_avals),
                in_names=tuple(all_in), out_names=tuple(out_names),
                lowering_input_output_aliases=(),
                sim_require_finite=False, sim_require_nnan=False, nc=nc)
            return tuple(outs)

        devices = jax.devices()[:NCORES]
        mesh = Mesh(np.asarray(devices), ("core",))
        sharded = jax.jit(
            shard_map(_body, mesh=mesh,
                      in_specs=(PartitionSpec("core"),) * (n_params + n_outs),
                      out_specs=(PartitionSpec("core"),) * n_outs,
                      check_rep=False),
            donate_argnums=tuple(range(n_params, n_params + n_outs)),
            keep_unused=True)
        _CACHE[key] = (sharded, in_names, out_names, out_avals, mesh)
    return _CACHE[key]


def run_device(in_maps, cap, g_off, TOTAL_G):
    import jax
    from jax.sharding import PartitionSpec
    sharded, in_names, out_names, out_avals, mesh = _get_compiled(
        cap, g_off, TOTAL_G)
    concat_in = [
        np.concatenate([np.asarray(in_maps[c][name]) for c in range(NCORES)],
                       axis=0)
        for name in in_names]
    concat_zeros = [
        np.zeros((NCORES * a.shape[0], *a.shape[1:]), a.dtype)
        for a in out_avals]
    out_arrs = sharded(*concat_in, *concat_zeros)
    jax.block_until_ready(out_arrs)
    res = [
        {name: np.asarray(out_arrs[i]).reshape(NCORES, *out_avals[i].shape)[c]
         for i, name in enumerate(out_names)}
        for c in range(NCORES)]
    return res


def kernel(x, W1, root1, b1, W2, root2, b2, edge_index, edge_type, batch):
    in_maps, hc = _host_prep(x, W1, root1, b1, W2, root2, b2,
                             edge_index, edge_type, batch)
    res = run_device(in_maps, hc["cap"], hc["g_off"], hc["TOTAL_G"])
    total = np.zeros((NGRAPH, H), dtype=np.float32)
    for k in range(NCORES):
        total += res[k]["pool_out"]
    return (total / hc["gcounts"][:, None]).astype(np.float32)

